# revision 1
# baseline (speedup 1.0000x reference)
"""Trainium2 Bass kernel for nn_ConcatenationAggregator.

For each review r:
    out[r] = relu(concat(review_vecs[r],
                         user_vecs[adj_u[r]][perm_u],
                         item_vecs[adj_i[r]][perm_i]) @ W)

Strategy (pure data-parallel over reviews, 8 NeuronCores):
  - Feature permutations are folded into W on the host (192x64 constant).
  - Row gathers use the GPSIMD `dma_gather` ucode (int16 indices, <=1024
    indices per call).  Since the tables exceed 32768 rows, the host sorts
    each core's reviews into 8 groups by (user-table 32K chunk, item-table
    32K chunk) so that rebased indices fit int16.  The group sort is just a
    relabeling of which review each (partition, column) slot processes; the
    host un-permutes the output.
  - The review stream is host-transposed into a feature-major, packed
    128-partition layout so it feeds the PE rhs directly; the output is
    produced transposed from PSUM and unpacked on the host.
  - Gathered rows are PE-transposed (user sub-tile -> PSUM partitions 0:64,
    item -> 64:128) giving a K=128 stacked rhs so one matmul covers the
    user+item contribution; a second K=64 matmul adds the review term.
  - This toolchain build enforces ONE sync-wait slot per instruction, so
    the emission order is software-pipelined (matmuls of chunk t before the
    transposes of chunk t+1, relus of chunk t after the copies of chunk
    t+1), discarded "header" transposes absorb gather-DMA waits, and the
    kernel-tail drain is split into single-wait drains.
"""

import os
import types

import numpy as np

import concourse.bacc as bacc
import concourse.bass as bass
import concourse.mybir as mybir
import concourse.tile as tile
from concourse.bass_utils import run_bass_kernel_spmd
from concourse.masks import make_identity
from concourse.vector_clock import ScopedClock, VectorClock

F32 = mybir.dt.float32
I16 = mybir.dt.int16

N_CORES = 8
D = 64
SUB = 128                  # reviews per sub-tile
MAX_S = 8                  # sub-tiles per chunk (<=1024 gather indices)
TCH = 32768                # table chunk (int16 index range)

N_REVIEWS = 1_000_000
N_USERS = 100_000
N_ITEMS = 50_000
RPC = N_REVIEWS // N_CORES


def _split_drain_and_barrier(self, tick_clock, wait_clock):
    """Replacement for TileContext._drain_and_barrier: the stock tail drain
    waits on every live proc semaphore at once, which overflows this
    toolchain's one-sync-wait-per-instruction limit.  Emit one drain per
    semaphore instead."""
    gc = tick_clock.global_clock
    ticks = list(gc)
    idxs = [i for i, t in enumerate(ticks) if t > 0]
    for i in idxs:
        sub = [0] * len(ticks)
        sub[i] = ticks[i]
        drain_inst = self.nc.sync.drain()
        wait_clock.add_sem_waits(
            drain_inst.ins, ScopedClock({None: VectorClock(sub)}))
    if not idxs:
        drain_inst = self.nc.sync.drain()
        wait_clock.add_sem_waits(
            drain_inst.ins, ScopedClock({None: VectorClock(ticks)}))
    self.nc.all_engine_barrier()
    assert self.sems is not None
    popped = self.nc._tile_sem_poison_stack.pop()
    assert popped is self._sem_poison
    self.nc.clear_and_free_semaphores(list(self.sems.allocated().values()))
    self.nc.all_engine_barrier()


def _chunk_list(s_per_group):
    """[(group, s_subtiles, row_base_slots, idxcol_base), ...] — shared by
    host packing and device program.  s values are even, <= MAX_S."""
    chunks = []
    row = 0
    col = 0
    for g, sg in enumerate(s_per_group):
        left = sg
        while left > 0:
            s = min(MAX_S, left)
            chunks.append((g, s, row, col))
            row += s * SUB
            col += s * 8
            left -= s
    return chunks


BUFS = int(os.environ.get("KBUFS", "3"))
PREF = int(os.environ.get("KPREF", "1"))


def _build_program(chunks, n_users, n_items):
    nc = bacc.Bacc("TRN2", target_bir_lowering=False, debug=False,
                   enable_asserts=False)
    padtot = sum(s for (_, s, _, _) in chunks) * SUB
    icols = padtot // 16

    rt_d = nc.dram_tensor("rt", [64, padtot], F32, kind="ExternalInput")
    uidx_d = nc.dram_tensor("uidx", [128, icols], I16, kind="ExternalInput")
    iidx_d = nc.dram_tensor("iidx", [128, icols], I16, kind="ExternalInput")
    tblu_d = nc.dram_tensor("tblu", [n_users, D], F32, kind="ExternalInput")
    tbli_d = nc.dram_tensor("tbli", [n_items, D], F32, kind="ExternalInput")
    w1_d = nc.dram_tensor("w1", [64, 64], F32, kind="ExternalInput")
    w2_d = nc.dram_tensor("w2p", [64, 64], F32, kind="ExternalInput")
    w3_d = nc.dram_tensor("w3p", [64, 64], F32, kind="ExternalInput")
    out_d = nc.dram_tensor("out2", [64, padtot], F32, kind="ExternalOutput")

    T = len(chunks)

    with tile.TileContext(nc) as tc:
        tc._drain_and_barrier = types.MethodType(_split_drain_and_barrier, tc)
        with tc.tile_pool(name="const", bufs=1) as constp, \
             tc.tile_pool(name="rtp", bufs=BUFS) as rtp, \
             tc.tile_pool(name="gup", bufs=BUFS) as gup, \
             tc.tile_pool(name="gip", bufs=BUFS) as gip, \
             tc.tile_pool(name="outp", bufs=BUFS) as outp, \
             tc.tile_pool(name="utp", bufs=BUFS) as utp, \
             tc.tile_pool(name="itp", bufs=BUFS) as itp, \
             tc.tile_pool(name="scr", bufs=1, space="PSUM") as scrp, \
             tc.tile_pool(name="tpp", bufs=4, space="PSUM") as tpp, \
             tc.tile_pool(name="mmp", bufs=3, space="PSUM") as mmp:

            ident = constp.tile([128, 128], F32)
            make_identity(nc, ident[:])
            w1_t = constp.tile([64, 64], F32)
            nc.sync.dma_start(out=w1_t[:], in_=w1_d.ap()[:, :])
            w2_t = constp.tile([64, 64], F32)
            nc.sync.dma_start(out=w2_t[:], in_=w2_d.ap()[:, :])
            w3_t = constp.tile([64, 64], F32)
            nc.sync.dma_start(out=w3_t[:], in_=w3_d.ap()[:, :])
            uidx_t = constp.tile([128, icols], I16)
            nc.sync.dma_start(out=uidx_t[:], in_=uidx_d.ap()[:, :])
            iidx_t = constp.tile([128, icols], I16)
            nc.sync.dma_start(out=iidx_t[:], in_=iidx_d.ap()[:, :])
            dummy_sb = constp.tile([64, 128], F32)
            pscr = constp.tile([16, 16], I16)

            scratch = scrp.tile([64, 128], F32)
            # PE warmups: observe the identity (Pool) and weight-load (HWDGE)
            # semaphores with one wait each.
            nc.tensor.transpose(out=scratch[:], in_=ident[:, 0:64],
                                identity=ident[:])
            nc.tensor.matmul(out=scratch[:, 0:64], lhsT=w2_t[:],
                             rhs=w2_t[:], start=True, stop=True)
            nc.tensor.matmul(out=scratch[:, 0:64], lhsT=w3_t[:],
                             rhs=w3_t[:], start=True, stop=True)
            nc.tensor.matmul(out=scratch[:, 0:64], lhsT=w1_t[:],
                             rhs=w1_t[:], start=True, stop=True)
            # Pool warmups: observe the index-table loads.
            nc.gpsimd.tensor_copy(out=pscr[:, :], in_=uidx_t[0:16, 0:16])
            nc.gpsimd.tensor_copy(out=pscr[:, :], in_=iidx_t[0:16, 0:16])

            nreg = {}
            for (_, s, _, _) in chunks:
                if s not in nreg:
                    nreg[s] = nc.gpsimd.to_reg(s * SUB)

            ubase = [g // 2 * TCH for g in range(8)]
            usize = [min(TCH, n_users - b) for b in ubase]
            ibase = [g % 2 * TCH for g in range(8)]
            isize = [min(TCH, n_items - b) for b in ibase]

            rt_tiles = [None] * T
            gu_tiles = [None] * T
            gi_tiles = [None] * T
            ui_tiles = [None] * T
            ps_tiles = [None] * T
            o_tiles = [None] * T

            def issue_loads(t):
                g, s, row, col = chunks[t]
                rt_t = rtp.tile([64, MAX_S * 128], F32, tag="rt")
                nc.sync.dma_start(
                    out=rt_t[:, :s * 128],
                    in_=rt_d.ap()[:, row: row + s * 128])
                gu_t = gup.tile([128, MAX_S * 64], F32, tag="gu")
                nc.gpsimd.dma_gather(
                    out_ap=gu_t[:, :s * 64].rearrange("p (n d) -> p n d", d=64),
                    in_ap=tblu_d.ap()[ubase[g]:ubase[g] + usize[g], :],
                    idxs_ap=uidx_t[:, col:col + s * 8],
                    num_idxs=s * SUB, num_idxs_reg=nreg[s], elem_size=64)
                gi_t = gip.tile([128, MAX_S * 64], F32, tag="gi")
                nc.gpsimd.dma_gather(
                    out_ap=gi_t[:, :s * 64].rearrange("p (n d) -> p n d", d=64),
                    in_ap=tbli_d.ap()[ibase[g]:ibase[g] + isize[g], :],
                    idxs_ap=iidx_t[:, col:col + s * 8],
                    num_idxs=s * SUB, num_idxs_reg=nreg[s], elem_size=64)
                rt_tiles[t], gu_tiles[t], gi_tiles[t] = rt_t, gu_t, gi_t

            def issue_transposes(t):
                _, s, _, _ = chunks[t]
                gu_t, gi_t = gu_tiles[t], gi_tiles[t]
                if os.environ.get("KHDR", "0") == "1":
                    # Discarded header transposes absorb the two gather waits.
                    nc.tensor.transpose(out=scratch[:], in_=gu_t[:, 0:64],
                                        identity=ident[:])
                    nc.tensor.transpose(out=scratch[:], in_=gi_t[:, 0:64],
                                        identity=ident[:])
                ut_t = utp.tile([64, MAX_S * 128], F32, tag="ut")
                it_t = itp.tile([64, MAX_S * 128], F32, tag="it")
                for g4 in range((s + 3) // 4):
                    w = min(4, s - g4 * 4)
                    tpu = tpp.tile([64, 512], F32, tag="tp")
                    for jj in range(w):
                        j = g4 * 4 + jj
                        nc.tensor.transpose(
                            out=tpu[:, jj * 128:(jj + 1) * 128],
                            in_=gu_t[:, j * 64:(j + 1) * 64],
                            identity=ident[:])
                    nc.vector.tensor_copy(
                        out=ut_t[:, g4 * 512:g4 * 512 + w * 128],
                        in_=tpu[:, :w * 128])
                    tpi = tpp.tile([64, 512], F32, tag="tp")
                    for jj in range(w):
                        j = g4 * 4 + jj
                        nc.tensor.transpose(
                            out=tpi[:, jj * 128:(jj + 1) * 128],
                            in_=gi_t[:, j * 64:(j + 1) * 64],
                            identity=ident[:])
                    nc.vector.tensor_copy(
                        out=it_t[:, g4 * 512:g4 * 512 + w * 128],
                        in_=tpi[:, :w * 128])
                ui_tiles[t] = (ut_t, it_t)

            def issue_matmuls(t):
                _, s, _, _ = chunks[t]
                n = s * 64
                pss = []
                rt_t = rt_tiles[t]
                ut_t, it_t = ui_tiles[t]
                for q in range(2):
                    ps = mmp.tile([64, 512], F32, tag="mm")
                    ps_s = ps[:, :n]
                    nc.tensor.matmul(out=ps_s, lhsT=w2_t[:],
                                     rhs=ut_t[:, q * n:(q + 1) * n],
                                     start=True, stop=False)
                    nc.tensor.matmul(out=ps_s, lhsT=w3_t[:],
                                     rhs=it_t[:, q * n:(q + 1) * n],
                                     start=False, stop=False)
                    nc.tensor.matmul(out=ps_s, lhsT=w1_t[:],
                                     rhs=rt_t[:, q * n:(q + 1) * n],
                                     start=False, stop=True)
                    pss.append(ps)
                ps_tiles[t] = pss

            def issue_relus(t):
                _, s, _, _ = chunks[t]
                n = s * 64
                pss = ps_tiles[t]
                o_t = outp.tile([64, MAX_S * 128], F32, tag="o")
                for q in range(2):
                    nc.vector.tensor_scalar_max(
                        out=o_t[:, q * n:(q + 1) * n],
                        in0=pss[q][:, :n], scalar1=0.0)
                o_tiles[t] = o_t

            def issue_store(t):
                _, s, row, _ = chunks[t]
                nc.sync.dma_start(
                    out=out_d.ap()[:, row: row + s * 128],
                    in_=o_tiles[t][:, :s * 128])

            # Software-pipelined emission (see module docstring).
            for tt in range(min(PREF, T)):
                issue_loads(tt)
            issue_transposes(0)
            for t in range(T):
                if t + PREF < T:
                    issue_loads(t + PREF)
                issue_matmuls(t)
                if t + 1 < T:
                    issue_transposes(t + 1)
                else:
                    # Dummy PE op after the last matmuls + a DVE observer so
                    # the last relus elide their PE wait.
                    nc.tensor.transpose(out=scratch[:], in_=ident[:, 0:64],
                                        identity=ident[:])
                    nc.vector.tensor_copy(out=dummy_sb[:], in_=scratch[:])
                issue_relus(t)
                issue_store(t)
    nc.finalize()
    return nc


_PROGRAM_CACHE: dict = {}


def _get_program(chunk_key, n_users, n_items):
    key = (chunk_key, n_users, n_items)
    if key not in _PROGRAM_CACHE:
        _PROGRAM_CACHE[key] = (
            _build_program(_chunk_list(list(chunk_key)), n_users, n_items))
    return _PROGRAM_CACHE[key]


def _pack_rt(rv_sorted, chunks):
    """[PADTOT, 64] sorted/padded reviews -> [64, PADTOT] feature-major."""
    return np.ascontiguousarray(rv_sorted.T)


def _unpack_out(o2, chunks):
    """[64, PADTOT] transposed output -> [PADTOT, 64]."""
    return np.ascontiguousarray(o2.T)


def _wrap_idx(flat_sorted, chunks):
    """Rebased int16 indices [PADTOT] -> [128, PADTOT//16] in dma_gather's
    wrapped layout: per chunk block [128, 8*s] with block[p, m] =
    flat[m*16 + p%16], replicated across the 8 16-partition groups."""
    cols = []
    for (_, s, row, _) in chunks:
        blk = flat_sorted[row:row + s * SUB].reshape(s * 8, 16).T  # [16, 8s]
        cols.append(np.tile(blk, (8, 1)))
    return np.ascontiguousarray(np.concatenate(cols, axis=1))


def _run(review_vecs, user_vecs, item_vecs, W,
         review_user_adj, review_item_adj, perm_u, perm_i,
         n_cores, rpc):
    n_users = user_vecs.shape[0]
    n_items = item_vecs.shape[0]

    W = np.asarray(W, np.float32)
    W1 = np.ascontiguousarray(W[0:64])
    W2 = W[64:128]
    W3 = W[128:192]
    perm_u = np.asarray(perm_u, np.int64)
    perm_i = np.asarray(perm_i, np.int64)
    W2p = np.empty_like(W2)
    W2p[perm_u] = W2
    W3p = np.empty_like(W3)
    W3p[perm_i] = W3
    W2p = np.ascontiguousarray(W2p)
    W3p = np.ascontiguousarray(W3p)

    user_vecs = np.ascontiguousarray(np.asarray(user_vecs, np.float32))
    item_vecs = np.ascontiguousarray(np.asarray(item_vecs, np.float32))
    review_vecs = np.asarray(review_vecs, np.float32)
    au_all = np.asarray(review_user_adj, np.int64)
    ai_all = np.asarray(review_item_adj, np.int64)

    # Group each core's reviews by (user 32K chunk, item 32K chunk).
    per_core = []
    s_max = np.zeros(8, np.int64)
    for c in range(n_cores):
        lo = c * rpc
        au = au_all[lo:lo + rpc]
        ai = ai_all[lo:lo + rpc]
        grp = (au // TCH) * 2 + (ai // TCH)
        order = np.argsort(grp, kind="stable")
        counts = np.bincount(grp, minlength=8)
        per_core.append((order, counts))
        s_max = np.maximum(s_max, -(-counts // SUB))
    # shared chunk structure: even sub-tile counts per group
    s_per_group = [int(s + (s % 2)) for s in s_max]
    chunk_key = tuple(s_per_group)
    chunks = _chunk_list(s_per_group)
    padtot = sum(s for (_, s, _, _) in chunks) * SUB

    nc = _get_program(chunk_key, n_users, n_items)

    in_maps = []
    slotmaps = []
    for c in range(n_cores):
        lo = c * rpc
        au = au_all[lo:lo + rpc]
        ai = ai_all[lo:lo + rpc]
        order, counts = per_core[c]
        grp_sorted_bounds = np.cumsum(counts)
        slotmap = np.full(padtot, -1, np.int64)
        row = 0
        start = 0
        for g in range(8):
            cnt = int(counts[g])
            ids = order[start:start + cnt]
            slotmap[row:row + cnt] = ids
            start += cnt
            row += s_per_group[g] * SUB
        valid = slotmap >= 0
        sl = np.where(valid, slotmap, 0)

        rv_sorted = np.where(valid[:, None],
                             review_vecs[lo:lo + rpc][sl], 0.0).astype(np.float32)
        slot_g = np.repeat(np.arange(8), np.array(s_per_group) * SUB)
        u_reb = np.where(valid, au[sl] - (slot_g // 2) * TCH, 0).astype(np.int16)
        i_reb = np.where(valid, ai[sl] - (slot_g % 2) * TCH, 0).astype(np.int16)

        in_maps.append({
            "rt": _pack_rt(rv_sorted, chunks),
            "uidx": _wrap_idx(u_reb, chunks),
            "iidx": _wrap_idx(i_reb, chunks),
            "tblu": user_vecs,
            "tbli": item_vecs,
            "w1": W1,
            "w2p": W2p,
            "w3p": W3p,
        })
        slotmaps.append((slotmap, valid))

    res = run_bass_kernel_spmd(nc, in_maps, core_ids=list(range(n_cores)))

    out = np.empty((n_cores * rpc, 64), np.float32)
    for c in range(n_cores):
        o2 = np.asarray(res.results[c]["out2"], np.float32)
        out_sorted = _unpack_out(o2, chunks)
        slotmap, valid = slotmaps[c]
        out[c * rpc + slotmap[valid]] = out_sorted[valid]
    return out


def kernel(**inputs) -> np.ndarray:
    return _run(
        inputs["review_vecs"], inputs["user_vecs"], inputs["item_vecs"],
        inputs["W"], inputs["review_user_adj"], inputs["review_item_adj"],
        inputs["perm_u"], inputs["perm_i"],
        n_cores=N_CORES, rpc=RPC)



# revision 4
# speedup vs baseline: 1.8533x; 1.8533x over previous
"""Trainium2 Bass kernel for nn_ConcatenationAggregator.

For each review r:
    out[r] = relu(concat(review_vecs[r],
                         user_vecs[adj_u[r]][perm_u],
                         item_vecs[adj_i[r]][perm_i]) @ W)

Strategy (pure data-parallel over reviews, 8 NeuronCores):
  - Feature permutations are folded into W on the host (192x64 constant).
  - Tables are stored hi/lo-split in bf16: row = [bf16(x) | bf16(x - hi)],
    256 bytes.  A transpose-mode GPSIMD dma_gather delivers each gathered
    row as a COLUMN across 128 partitions (hi in partitions 0:64, lo in
    64:128), i.e. already in matmul-rhs layout - no PE transposes needed.
    With lhsT = [W; W] the matmul computes W^T (hi + lo) = full-precision
    row dot products (bf16 rounding only on W and the review stream).
  - Review stream and output travel as bf16, host-transposed feature-major.
  - dma_gather needs int16 indices; tables exceed 32768 rows, so the host
    sorts each core's reviews into 8 groups by (user-table 32K chunk,
    item-table 32K chunk) and rebases indices.  The host un-permutes the
    output afterwards.
  - Per 1024-review chunk: 1 rt DMA + 2 gathers (1024 idxs each, the SWDGE
    ring cap) + per 512-col PSUM tile 3 accumulating bf16 matmuls
    (K=128 user, K=128 item, K=64 review) + DVE relu to bf16 + 1 out DMA.
  - This toolchain build enforces ONE sync-wait slot per instruction, so
    pool-reuse waits are absorbed by tiny single-purpose ops: a 1-column
    PE matmul "warms" each PSUM tile (carrying the relu-freed wait) and a
    1-element DVE memset warms each out tile (carrying the store-freed
    wait); the kernel-tail drain is split into single-wait drains.
"""

import os
import types

import numpy as np
import ml_dtypes

import concourse.bacc as bacc
import concourse.bass as bass
import concourse.mybir as mybir
import concourse.tile as tile
from concourse.bass_utils import run_bass_kernel_spmd
from concourse.vector_clock import ScopedClock, VectorClock

F32 = mybir.dt.float32
BF16 = mybir.dt.bfloat16
I16 = mybir.dt.int16

N_CORES = 8
D = 64
SUB = 128                  # reviews per sub-tile
MAX_S = 7                  # sub-tiles per chunk (transpose dma_gather stages
                           # idx*256B through the 16KB DMA scratch: 896 max)
TCH = 32768                # table chunk (int16 index range)
PCOL = 512                 # PSUM tile columns

N_REVIEWS = 1_000_000
N_USERS = 100_000
N_ITEMS = 50_000
RPC = N_REVIEWS // N_CORES

BUFS = int(os.environ.get("KBUFS", "3"))
PREF = int(os.environ.get("KPREF", "2"))
PSUM_BUFS = int(os.environ.get("KPSUM", "6"))


def _split_drain_and_barrier(self, tick_clock, wait_clock):
    """Replacement for TileContext._drain_and_barrier: the stock tail drain
    waits on every live proc semaphore at once, which overflows this
    toolchain's one-sync-wait-per-instruction limit.  Emit one drain per
    semaphore instead."""
    gc = tick_clock.global_clock
    ticks = list(gc)
    idxs = [i for i, t in enumerate(ticks) if t > 0]
    for i in idxs:
        sub = [0] * len(ticks)
        sub[i] = ticks[i]
        drain_inst = self.nc.sync.drain()
        wait_clock.add_sem_waits(
            drain_inst.ins, ScopedClock({None: VectorClock(sub)}))
    if not idxs:
        drain_inst = self.nc.sync.drain()
        wait_clock.add_sem_waits(
            drain_inst.ins, ScopedClock({None: VectorClock(ticks)}))
    self.nc.all_engine_barrier()
    assert self.sems is not None
    popped = self.nc._tile_sem_poison_stack.pop()
    assert popped is self._sem_poison
    self.nc.clear_and_free_semaphores(list(self.sems.allocated().values()))
    self.nc.all_engine_barrier()


def _chunk_list(s_per_group):
    """[(group, s_subtiles, row_base_slots, idxcol_base), ...] — shared by
    host packing and device program.  s values <= MAX_S."""
    chunks = []
    row = 0
    col = 0
    for g, sg in enumerate(s_per_group):
        left = sg
        while left > 0:
            s = min(MAX_S, left)
            chunks.append((g, s, row, col))
            row += s * SUB
            col += s * 8
            left -= s
    return chunks


def _build_program(chunks, n_users, n_items):
    nc = bacc.Bacc("TRN2", target_bir_lowering=False, debug=False,
                   enable_asserts=False)
    padtot = sum(s for (_, s, _, _) in chunks) * SUB
    icols = padtot // 16

    rt_d = nc.dram_tensor("rt", [64, padtot], BF16, kind="ExternalInput")
    uidx_d = nc.dram_tensor("uidx", [128, icols], I16, kind="ExternalInput")
    iidx_d = nc.dram_tensor("iidx", [128, icols], I16, kind="ExternalInput")
    tblu_d = nc.dram_tensor("tblu", [n_users, 128], BF16, kind="ExternalInput")
    tbli_d = nc.dram_tensor("tbli", [n_items, 128], BF16, kind="ExternalInput")
    w1_d = nc.dram_tensor("w1", [64, 64], BF16, kind="ExternalInput")
    w2_d = nc.dram_tensor("w2d", [128, 64], BF16, kind="ExternalInput")
    w3_d = nc.dram_tensor("w3d", [128, 64], BF16, kind="ExternalInput")
    out_d = nc.dram_tensor("out2", [64, padtot], BF16, kind="ExternalOutput")

    T = len(chunks)

    with tile.TileContext(nc) as tc:
        tc._drain_and_barrier = types.MethodType(_split_drain_and_barrier, tc)
        with tc.tile_pool(name="const", bufs=1) as constp, \
             tc.tile_pool(name="rtp", bufs=BUFS) as rtp, \
             tc.tile_pool(name="gup", bufs=BUFS) as gup, \
             tc.tile_pool(name="gip", bufs=BUFS) as gip, \
             tc.tile_pool(name="outp", bufs=BUFS) as outp, \
             tc.tile_pool(name="scr", bufs=1, space="PSUM") as scrp, \
             tc.tile_pool(name="mmp", bufs=PSUM_BUFS, space="PSUM") as mmp:

            # Index tables first so the weight loads are the newest HWDGE
            # ticks (PE warmup then covers all weight loads with one wait).
            uidx_t = constp.tile([128, icols], I16)
            nc.sync.dma_start(out=uidx_t[:], in_=uidx_d.ap()[:, :])
            iidx_t = constp.tile([128, icols], I16)
            nc.sync.dma_start(out=iidx_t[:], in_=iidx_d.ap()[:, :])
            w1_t = constp.tile([64, 64], BF16)
            nc.sync.dma_start(out=w1_t[:], in_=w1_d.ap()[:, :])
            w2_t = constp.tile([128, 64], BF16)
            nc.sync.dma_start(out=w2_t[:], in_=w2_d.ap()[:, :])
            w3_t = constp.tile([128, 64], BF16)
            nc.sync.dma_start(out=w3_t[:], in_=w3_d.ap()[:, :])
            pscr = constp.tile([16, 16], I16)

            scratch = scrp.tile([64, PCOL], F32)
            # PE warmups: observe the weight-load (HWDGE) sem, ramp p-state.
            nc.tensor.matmul(out=scratch[:, 0:64], lhsT=w1_t[:],
                             rhs=w1_t[:], start=True, stop=True)
            nc.tensor.matmul(out=scratch[:, 0:64], lhsT=w2_t[:],
                             rhs=w2_t[:, 0:64], start=True, stop=True)
            nc.tensor.matmul(out=scratch[:, 0:64], lhsT=w3_t[:],
                             rhs=w3_t[:, 0:64], start=True, stop=True)
            # Pool warmups: observe the index-table loads with single waits.
            nc.gpsimd.tensor_copy(out=pscr[:, :], in_=uidx_t[0:16, 0:16])
            nc.gpsimd.tensor_copy(out=pscr[:, :], in_=iidx_t[0:16, 0:16])

            nreg = {}
            for (_, s, _, _) in chunks:
                if s not in nreg:
                    nreg[s] = nc.gpsimd.to_reg(s * SUB)

            ubase = [g // 2 * TCH for g in range(8)]
            usize = [min(TCH, n_users - b) for b in ubase]
            ibase = [g % 2 * TCH for g in range(8)]
            isize = [min(TCH, n_items - b) for b in ibase]

            rt_tiles = [None] * T
            gu_tiles = [None] * T
            gi_tiles = [None] * T
            ps_tiles = [None] * T
            o_tiles = [None] * T

            def issue_loads(t):
                g, s, row, col = chunks[t]
                rt_t = rtp.tile([64, MAX_S * 128], BF16, tag="rt")
                nc.sync.dma_start(
                    out=rt_t[:, :s * 128],
                    in_=rt_d.ap()[:, row: row + s * 128])
                gu_t = gup.tile([128, MAX_S * 128], BF16, tag="gu")
                nc.gpsimd.dma_gather(
                    out_ap=gu_t[:, :s * 128].rearrange(
                        "p (n i) -> p n i", n=1),
                    in_ap=tblu_d.ap()[ubase[g]:ubase[g] + usize[g], :],
                    idxs_ap=uidx_t[:, col:col + s * 8],
                    num_idxs=s * SUB, num_idxs_reg=nreg[s],
                    elem_size=128, transpose=True)
                gi_t = gip.tile([128, MAX_S * 128], BF16, tag="gi")
                nc.gpsimd.dma_gather(
                    out_ap=gi_t[:, :s * 128].rearrange(
                        "p (n i) -> p n i", n=1),
                    in_ap=tbli_d.ap()[ibase[g]:ibase[g] + isize[g], :],
                    idxs_ap=iidx_t[:, col:col + s * 8],
                    num_idxs=s * SUB, num_idxs_reg=nreg[s],
                    elem_size=128, transpose=True)
                rt_tiles[t], gu_tiles[t], gi_tiles[t] = rt_t, gu_t, gi_t

            def issue_matmuls(t):
                _, s, _, _ = chunks[t]
                rt_t, gu_t, gi_t = rt_tiles[t], gu_tiles[t], gi_tiles[t]
                nq = (s * 128 + PCOL - 1) // PCOL
                pss = []
                for q in range(nq):
                    n = min(PCOL, s * 128 - q * PCOL)
                    ps = mmp.tile([64, PCOL], F32, tag="mm")
                    # 1-column const warm write: absorbs the PSUM-reuse wait
                    # (relu of an earlier chunk) so the first real matmul
                    # keeps its single wait slot for the gather sem.
                    nc.tensor.matmul(out=ps[0:1, 0:1], lhsT=w2_t[:, 0:1],
                                     rhs=w2_t[:, 0:1], start=True, stop=True)
                    ps_s = ps[:, :n]
                    lo = q * PCOL
                    nc.tensor.matmul(out=ps_s, lhsT=w2_t[:],
                                     rhs=gu_t[:, lo:lo + n],
                                     start=True, stop=False)
                    nc.tensor.matmul(out=ps_s, lhsT=w3_t[:],
                                     rhs=gi_t[:, lo:lo + n],
                                     start=False, stop=False)
                    nc.tensor.matmul(out=ps_s, lhsT=w1_t[:],
                                     rhs=rt_t[:, lo:lo + n],
                                     start=False, stop=True)
                    pss.append(ps)
                ps_tiles[t] = pss

            def issue_relus(t):
                _, s, _, _ = chunks[t]
                pss = ps_tiles[t]
                o_t = outp.tile([64, MAX_S * 128], BF16, tag="o")
                # 1-element warm write: absorbs the out-tile-reuse wait
                # (store of an earlier chunk).
                nc.vector.memset(o_t[0:1, 0:1], 0)
                for q, ps in enumerate(pss):
                    n = min(PCOL, s * 128 - q * PCOL)
                    nc.vector.tensor_scalar_max(
                        out=o_t[:, q * PCOL:q * PCOL + n],
                        in0=ps[:, :n], scalar1=0.0)
                o_tiles[t] = o_t

            def issue_store(t):
                _, s, row, _ = chunks[t]
                nc.sync.dma_start(
                    out=out_d.ap()[:, row: row + s * 128],
                    in_=o_tiles[t][:, :s * 128])

            # Software-pipelined emission.
            for tt in range(min(PREF, T)):
                issue_loads(tt)
            for t in range(T):
                if t + PREF < T:
                    issue_loads(t + PREF)
                issue_matmuls(t)
                issue_relus(t)
                issue_store(t)
    nc.finalize()
    return nc


_PROGRAM_CACHE: dict = {}


def _get_program(chunk_key, n_users, n_items):
    key = (chunk_key, n_users, n_items)
    if key not in _PROGRAM_CACHE:
        _PROGRAM_CACHE[key] = (
            _build_program(_chunk_list(list(chunk_key)), n_users, n_items))
    return _PROGRAM_CACHE[key]


def _wrap_idx(flat_sorted, chunks):
    """Rebased int16 indices [PADTOT] -> [128, PADTOT//16] in dma_gather's
    wrapped layout: per chunk block [128, 8*s] with block[p, m] =
    flat[m*16 + p%16], replicated across the 8 16-partition groups."""
    cols = []
    for (_, s, row, _) in chunks:
        blk = flat_sorted[row:row + s * SUB].reshape(s * 8, 16).T  # [16, 8s]
        cols.append(np.tile(blk, (8, 1)))
    return np.ascontiguousarray(np.concatenate(cols, axis=1))


def _hilo(tbl):
    """[N, 64] f32 -> [N, 128] bf16 rows [hi | lo], hi + lo == x exactly
    up to bf16(residual) rounding."""
    hi = tbl.astype(ml_dtypes.bfloat16)
    lo = (tbl - hi.astype(np.float32)).astype(ml_dtypes.bfloat16)
    return np.ascontiguousarray(np.concatenate([hi, lo], axis=1))


def _run(review_vecs, user_vecs, item_vecs, W,
         review_user_adj, review_item_adj, perm_u, perm_i,
         n_cores, rpc):
    n_users = user_vecs.shape[0]
    n_items = item_vecs.shape[0]

    W = np.asarray(W, np.float32)
    W1 = W[0:64]
    W2 = W[64:128]
    W3 = W[128:192]
    perm_u = np.asarray(perm_u, np.int64)
    perm_i = np.asarray(perm_i, np.int64)
    W2p = np.empty_like(W2)
    W2p[perm_u] = W2
    W3p = np.empty_like(W3)
    W3p[perm_i] = W3
    w1b = np.ascontiguousarray(W1.astype(ml_dtypes.bfloat16))
    w2d = np.ascontiguousarray(
        np.concatenate([W2p, W2p], axis=0).astype(ml_dtypes.bfloat16))
    w3d = np.ascontiguousarray(
        np.concatenate([W3p, W3p], axis=0).astype(ml_dtypes.bfloat16))

    user_vecs = np.asarray(user_vecs, np.float32)
    item_vecs = np.asarray(item_vecs, np.float32)
    tblu = _hilo(user_vecs)
    tbli = _hilo(item_vecs)
    review_vecs = np.asarray(review_vecs, np.float32)
    au_all = np.asarray(review_user_adj, np.int64)
    ai_all = np.asarray(review_item_adj, np.int64)

    # Group each core's reviews by (user 32K chunk, item 32K chunk).
    per_core = []
    s_max = np.zeros(8, np.int64)
    for c in range(n_cores):
        lo = c * rpc
        au = au_all[lo:lo + rpc]
        ai = ai_all[lo:lo + rpc]
        grp = (au // TCH) * 2 + (ai // TCH)
        order = np.argsort(grp, kind="stable")
        counts = np.bincount(grp, minlength=8)
        per_core.append((order, counts))
        s_max = np.maximum(s_max, -(-counts // SUB))
    # shared chunk structure: sub-tile counts padded to multiples of 4 so
    # every chunk is a whole number of 512-col PSUM tiles
    s_per_group = [int(-(-s // 4) * 4) for s in s_max]
    chunk_key = tuple(s_per_group)
    chunks = _chunk_list(s_per_group)
    padtot = sum(s for (_, s, _, _) in chunks) * SUB

    nc = _get_program(chunk_key, n_users, n_items)

    in_maps = []
    slotmaps = []
    for c in range(n_cores):
        lo = c * rpc
        au = au_all[lo:lo + rpc]
        ai = ai_all[lo:lo + rpc]
        order, counts = per_core[c]
        slotmap = np.full(padtot, -1, np.int64)
        row = 0
        start = 0
        for g in range(8):
            cnt = int(counts[g])
            ids = order[start:start + cnt]
            slotmap[row:row + cnt] = ids
            start += cnt
            row += s_per_group[g] * SUB
        valid = slotmap >= 0
        sl = np.where(valid, slotmap, 0)

        rv_sorted = np.where(
            valid[:, None], review_vecs[lo:lo + rpc][sl], 0.0)
        rt2 = np.ascontiguousarray(
            rv_sorted.T.astype(ml_dtypes.bfloat16))
        slot_g = np.repeat(np.arange(8), np.array(s_per_group) * SUB)
        u_reb = np.where(valid, au[sl] - (slot_g // 2) * TCH, 0).astype(
            np.int16)
        i_reb = np.where(valid, ai[sl] - (slot_g % 2) * TCH, 0).astype(
            np.int16)

        in_maps.append({
            "rt": rt2,
            "uidx": _wrap_idx(u_reb, chunks),
            "iidx": _wrap_idx(i_reb, chunks),
            "tblu": tblu,
            "tbli": tbli,
            "w1": w1b,
            "w2d": w2d,
            "w3d": w3d,
        })
        slotmaps.append((slotmap, valid))

    res = run_bass_kernel_spmd(nc, in_maps, core_ids=list(range(n_cores)))

    out = np.empty((n_cores * rpc, 64), np.float32)
    for c in range(n_cores):
        o2 = np.asarray(res.results[c]["out2"]).astype(np.float32)
        out_sorted = o2.T
        slotmap, valid = slotmaps[c]
        out[c * rpc + slotmap[valid]] = out_sorted[valid]
    return out


def kernel(**inputs) -> np.ndarray:
    return _run(
        inputs["review_vecs"], inputs["user_vecs"], inputs["item_vecs"],
        inputs["W"], inputs["review_user_adj"], inputs["review_item_adj"],
        inputs["perm_u"], inputs["perm_i"],
        n_cores=N_CORES, rpc=RPC)


# revision 12
# speedup vs baseline: 2.3551x; 1.2708x over previous
"""Trainium2 Bass kernel for nn_ConcatenationAggregator.

For each review r:
    out[r] = relu(concat(review_vecs[r],
                         user_vecs[adj_u[r]][perm_u],
                         item_vecs[adj_i[r]][perm_i]) @ W)

Strategy (data-parallel over reviews, 8 NeuronCores):
  - Feature permutations are folded into W on the host (192x64 constant).
  - USER side: hi/lo-split bf16 table rows [bf16(x) | bf16(x - hi)] (256B)
    gathered by a transpose-mode GPSIMD dma_gather that delivers each row
    as a COLUMN across 128 partitions - already in matmul-rhs layout.
    lhsT = [W2p; W2p] makes the matmul compute W^T (hi + lo).
  - ITEM side: gather-free.  The device first computes T = item_vecs @ W3p
    (transform phase: stream item^T, 391 window matmuls, copy to an
    SBUF-resident bf16 table T_sb[p, w*64+j] = T[w*128+p, j]).  Reviews are
    globally sorted by (user-group, item-window); each 128-id item window
    becomes one accumulating matmul with lhsT = T_sb window and rhs = a
    streamed fp8 one-hot matrix (0/1 exact in fp8, 128B/review, dense DMA
    at full bandwidth - 4x cheaper than a 256B gather descriptor).
  - ONE program for all 8 cores: reviews in every (group, window) cell are
    dealt round-robin to cores, so per-core cell counts are the shared
    ceil(n_cell/8) - column layout and window-matmul ranges are identical
    across cores.  Padded slots have all-zero one-hot columns.
  - dma_gather needs int16 indices: 4 user groups of 32768 rows; the host
    rebases.  Transpose-mode gathers stage idx*256B through the 16KB DMA
    scratch -> max 896 indices per call (MAX_S = 7).
  - Review stream and output travel as bf16, host-transposed feature-major.
  - This toolchain build enforces ONE sync-wait slot per instruction:
    pool-reuse waits are absorbed by 1-column PE matmul warms / 1-element
    DVE memsets, and the kernel-tail drain is split into single-wait drains.
"""

import os
import types

import numpy as np
import ml_dtypes

import concourse.bacc as bacc
import concourse.bass as bass
import concourse.mybir as mybir
import concourse.tile as tile
from concourse.bass_utils import run_bass_kernel_spmd
from concourse.vector_clock import ScopedClock, VectorClock

F32 = mybir.dt.float32
BF16 = mybir.dt.bfloat16
FP8 = mybir.dt.float8e4
I16 = mybir.dt.int16

N_CORES = 8
D = 64
SUB = 128                  # reviews per sub-tile
MAX_S = 7                  # sub-tiles per chunk (transpose dma_gather stages
                           # idx*256B through the 16KB DMA scratch: 896 max)
TCH = 32768                # user table chunk (int16 index range)
PCOL = 512                 # PSUM tile columns
WIN = 128                  # item ids per window
NGRP = 4                   # user groups

N_REVIEWS = 1_000_000
N_USERS = 100_000
N_ITEMS = 50_000
RPC = N_REVIEWS // N_CORES
NWIN = -(-N_ITEMS // WIN)  # 391
ITEM_PAD = -(-NWIN * WIN // 1024) * 1024  # 50176 (transform pieces of 1024)

BUFS = int(os.environ.get("KBUFS", "3"))
PREF = int(os.environ.get("KPREF", "2"))
PSUM_BUFS = int(os.environ.get("KPSUM", "5"))


def _split_drain_and_barrier(self, tick_clock, wait_clock):
    """Replacement for TileContext._drain_and_barrier: the stock tail drain
    waits on every live proc semaphore at once, which overflows this
    toolchain's one-sync-wait-per-instruction limit.  Emit one drain per
    semaphore instead."""
    gc = tick_clock.global_clock
    ticks = list(gc)
    idxs = [i for i, t in enumerate(ticks) if t > 0]
    for i in idxs:
        sub = [0] * len(ticks)
        sub[i] = ticks[i]
        drain_inst = self.nc.sync.drain()
        wait_clock.add_sem_waits(
            drain_inst.ins, ScopedClock({None: VectorClock(sub)}))
    if not idxs:
        drain_inst = self.nc.sync.drain()
        wait_clock.add_sem_waits(
            drain_inst.ins, ScopedClock({None: VectorClock(ticks)}))
    self.nc.all_engine_barrier()
    assert self.sems is not None
    popped = self.nc._tile_sem_poison_stack.pop()
    assert popped is self._sem_poison
    self.nc.clear_and_free_semaphores(list(self.sems.allocated().values()))
    self.nc.all_engine_barrier()


class Layout:
    """Shared (all-cores) column layout derived from global cell counts.

    m[g, w]: padded per-core review count of cell (user-group g, window w).
    Group sections are padded to multiples of SUB; chunks of <= MAX_S
    sub-tiles tile each group section.
    """

    def __init__(self, m):
        self.m = m  # [NGRP, NWIN] ints
        self.cell_base = np.zeros((NGRP, NWIN), np.int64)
        self.grp_base = np.zeros(NGRP, np.int64)
        self.grp_size = np.zeros(NGRP, np.int64)  # padded
        pos = 0
        for g in range(NGRP):
            self.grp_base[g] = pos
            csum = 0
            for w in range(NWIN):
                self.cell_base[g, w] = pos + csum
                csum += m[g, w]
            padded = -(-csum // SUB) * SUB
            self.grp_size[g] = padded
            pos += padded
        self.padtot = int(pos)
        # chunks: (g, s, row)
        self.chunks = []
        for g in range(NGRP):
            left = int(self.grp_size[g]) // SUB
            row = int(self.grp_base[g])
            while left > 0:
                s = min(MAX_S, left)
                self.chunks.append((g, s, row))
                row += s * SUB
                left -= s
        # per chunk: list of psum tiles [(lo, hi)], and per tile the window
        # matmul splits [(lo, hi, w)] in ABSOLUTE columns
        self.tiles = []
        for (g, s, row) in self.chunks:
            tl = []
            for q in range(-(-s * 128 // PCOL)):
                lo = row + q * PCOL
                hi = min(row + s * 128, lo + PCOL)
                wmms = []
                for w in range(NWIN):
                    if self.m[g, w] == 0:
                        continue
                    clo = int(self.cell_base[g, w])
                    chi = clo + int(self.m[g, w])
                    a, b = max(clo, lo), min(chi, hi)
                    if a < b:
                        wmms.append((a, b, w))
                tl.append((lo, hi, wmms))
            self.tiles.append(tl)

    def key(self):
        return tuple(self.m.reshape(-1).tolist())


def _build_program(lay: Layout, n_users):
    nc = bacc.Bacc("TRN2", target_bir_lowering=False, debug=False,
                   enable_asserts=False)
    padtot = lay.padtot
    icols = padtot // 16

    rt_d = nc.dram_tensor("rt", [64, padtot], BF16, kind="ExternalInput")
    oh_d = nc.dram_tensor("oh", [128, padtot], FP8, kind="ExternalInput")
    uidx_d = nc.dram_tensor("uidx", [128, icols], I16, kind="ExternalInput")
    tblu_d = nc.dram_tensor("tblu", [n_users, 128], BF16,
                            kind="ExternalInput")
    itemt_d = nc.dram_tensor("itemt", [64, ITEM_PAD], BF16,
                             kind="ExternalInput")
    w1_d = nc.dram_tensor("w1", [64, 64], BF16, kind="ExternalInput")
    w2_d = nc.dram_tensor("w2d", [128, 64], BF16, kind="ExternalInput")
    w3_d = nc.dram_tensor("w3p", [64, 64], BF16, kind="ExternalInput")
    out_d = nc.dram_tensor("out2", [64, padtot], BF16, kind="ExternalOutput")

    T = len(lay.chunks)
    NPIECE = ITEM_PAD // 1024  # transform pieces of 8 windows

    with tile.TileContext(nc) as tc:
        tc._drain_and_barrier = types.MethodType(_split_drain_and_barrier, tc)
        with tc.tile_pool(name="const", bufs=1) as constp, \
             tc.tile_pool(name="itp", bufs=3) as itp, \
             tc.tile_pool(name="rtp", bufs=BUFS) as rtp, \
             tc.tile_pool(name="ohp", bufs=BUFS) as ohp, \
             tc.tile_pool(name="gup", bufs=BUFS) as gup, \
             tc.tile_pool(name="outp", bufs=BUFS) as outp, \
             tc.tile_pool(name="scr", bufs=1, space="PSUM") as scrp, \
             tc.tile_pool(name="tfp", bufs=2, space="PSUM") as tfp, \
             tc.tile_pool(name="mmp", bufs=PSUM_BUFS, space="PSUM") as mmp:

            uidx_t = constp.tile([128, icols], I16)
            nc.sync.dma_start(out=uidx_t[:], in_=uidx_d.ap()[:, :])
            w1_t = constp.tile([64, 64], BF16)
            nc.sync.dma_start(out=w1_t[:], in_=w1_d.ap()[:, :])
            w2_t = constp.tile([128, 64], BF16)
            nc.sync.dma_start(out=w2_t[:], in_=w2_d.ap()[:, :])
            w3_t = constp.tile([64, 64], BF16)
            nc.sync.dma_start(out=w3_t[:], in_=w3_d.ap()[:, :])
            tsb = constp.tile([128, (ITEM_PAD // 128) * 64], BF16)
            pscr = constp.tile([16, 16], I16)

            scratch = scrp.tile([64, PCOL], F32)
            # PE warmups: observe the weight loads (HWDGE sem), ramp p-state.
            nc.tensor.matmul(out=scratch[:, 0:64], lhsT=w1_t[:],
                             rhs=w1_t[:], start=True, stop=True)
            nc.tensor.matmul(out=scratch[:, 0:64], lhsT=w2_t[:],
                             rhs=w2_t[:, 0:64], start=True, stop=True)
            nc.tensor.matmul(out=scratch[:, 0:64], lhsT=w3_t[:],
                             rhs=w3_t[:], start=True, stop=True)
            # Pool warmup: observe the user index-table load.
            nc.gpsimd.tensor_copy(out=pscr[:, :], in_=uidx_t[0:16, 0:16])

            nreg = {}
            for (_, s, _) in lay.chunks:
                if s not in nreg:
                    nreg[s] = nc.gpsimd.to_reg(s * SUB)

            # ---- transform phase: T_sb[p, w*64+j] = (item @ W3p)[w*128+p, j]
            for pc in range(NPIECE):
                it_t = itp.tile([64, 1024], BF16, tag="it")
                nc.sync.dma_start(
                    out=it_t[:],
                    in_=itemt_d.ap()[:, pc * 1024:(pc + 1) * 1024])
                ps = tfp.tile([128, PCOL], F32, tag="tf")
                # 1-col warm write absorbs the PSUM-reuse wait.
                nc.tensor.matmul(out=ps[0:1, 0:1], lhsT=w3_t[:, 0:1],
                                 rhs=w3_t[:, 0:1], start=True, stop=True)
                for k in range(8):
                    nc.tensor.matmul(
                        out=ps[:, k * 64:(k + 1) * 64],
                        lhsT=it_t[:, k * 128:(k + 1) * 128],
                        rhs=w3_t[:], start=True, stop=True)
                nc.vector.tensor_copy(
                    out=tsb[:, pc * 512:(pc + 1) * 512], in_=ps[:])
            # PE observes T_sb (DVE sem) once: later window matmuls elide it.
            nc.tensor.matmul(out=scratch[:, 0:64], lhsT=tsb[:, 0:64],
                             rhs=w2_t[:, 0:64], start=True, stop=True)

            rt_tiles = [None] * T
            oh_tiles = [None] * T
            gu_tiles = [None] * T
            ps_tiles = [None] * T
            o_tiles = [None] * T

            def issue_loads(t):
                g, s, row = lay.chunks[t]
                col = row // 16
                rt_t = rtp.tile([64, MAX_S * 128], BF16, tag="rt")
                nc.sync.dma_start(
                    out=rt_t[:, :s * 128],
                    in_=rt_d.ap()[:, row: row + s * 128])
                oh_t = ohp.tile([128, MAX_S * 128], FP8, tag="oh")
                nc.sync.dma_start(
                    out=oh_t[:, :s * 128],
                    in_=oh_d.ap()[:, row: row + s * 128])
                gu_t = gup.tile([128, MAX_S * 128], BF16, tag="gu")
                nc.gpsimd.dma_gather(
                    out_ap=gu_t[:, :s * 128].rearrange(
                        "p (n i) -> p n i", n=1),
                    in_ap=tblu_d.ap()[g * TCH:min((g + 1) * TCH, n_users), :],
                    idxs_ap=uidx_t[:, col:col + s * 8],
                    num_idxs=s * SUB, num_idxs_reg=nreg[s],
                    elem_size=128, transpose=True)
                rt_tiles[t], oh_tiles[t], gu_tiles[t] = rt_t, oh_t, gu_t

            def issue_matmuls(t):
                g, s, row = lay.chunks[t]
                rt_t, oh_t, gu_t = rt_tiles[t], oh_tiles[t], gu_tiles[t]
                pss = []
                for (lo, hi, wmms) in lay.tiles[t]:
                    n = hi - lo
                    ps = mmp.tile([64, PCOL], F32, tag="mm")
                    nc.tensor.matmul(out=ps[0:1, 0:1], lhsT=w2_t[:, 0:1],
                                     rhs=w2_t[:, 0:1], start=True, stop=True)
                    ps_s = ps[:, :n]
                    c0 = lo - row
                    nc.tensor.matmul(out=ps_s, lhsT=w2_t[:],
                                     rhs=gu_t[:, c0:c0 + n],
                                     start=True, stop=False)
                    for (a, b, w) in wmms:
                        nc.tensor.matmul(
                            out=ps[:, a - lo:b - lo],
                            lhsT=tsb[:, w * 64:(w + 1) * 64],
                            rhs=oh_t[:, a - row:b - row],
                            start=False, stop=False)
                    # last: full-range stop so every column's accumulation
                    # group closes (rt's SP wait is covered by the oh wait).
                    nc.tensor.matmul(out=ps_s, lhsT=w1_t[:],
                                     rhs=rt_t[:, c0:c0 + n],
                                     start=False, stop=True)
                    pss.append(ps)
                ps_tiles[t] = pss

            def issue_relus(t):
                g, s, row = lay.chunks[t]
                o_t = outp.tile([64, MAX_S * 128], BF16, tag="o")
                nc.vector.memset(o_t[0:1, 0:1], 0)
                for q, ps in enumerate(ps_tiles[t]):
                    lo, hi, _ = lay.tiles[t][q]
                    n = hi - lo
                    nc.vector.tensor_scalar_max(
                        out=o_t[:, lo - row:lo - row + n],
                        in0=ps[:, :n], scalar1=0.0)
                o_tiles[t] = o_t

            def issue_store(t):
                g, s, row = lay.chunks[t]
                nc.sync.dma_start(
                    out=out_d.ap()[:, row: row + s * 128],
                    in_=o_tiles[t][:, :s * 128])

            for tt in range(min(PREF, T)):
                issue_loads(tt)
            for t in range(T):
                if t + PREF < T:
                    issue_loads(t + PREF)
                issue_matmuls(t)
                issue_relus(t)
                issue_store(t)
    nc.finalize()
    return nc


_PROGRAM_CACHE: dict = {}


def _get_program(lay: Layout, n_users):
    key = (lay.key(), n_users)
    if key not in _PROGRAM_CACHE:
        _PROGRAM_CACHE[key] = _build_program(lay, n_users)
    return _PROGRAM_CACHE[key]


def _wrap_idx(flat_sorted, lay):
    """Rebased int16 indices [PADTOT] -> [128, PADTOT//16] in dma_gather's
    wrapped layout (per chunk: [p, m] = flat[m*16 + p%16], replicated)."""
    cols = []
    for (_, s, row) in lay.chunks:
        blk = flat_sorted[row:row + s * SUB].reshape(s * 8, 16).T
        cols.append(np.tile(blk, (8, 1)))
    return np.ascontiguousarray(np.concatenate(cols, axis=1))


def _hilo(tbl):
    """[N, 64] f32 -> [N, 128] bf16 rows [hi | lo]."""
    hi = tbl.astype(ml_dtypes.bfloat16)
    lo = (tbl - hi.astype(np.float32)).astype(ml_dtypes.bfloat16)
    return np.ascontiguousarray(np.concatenate([hi, lo], axis=1))


def _run(review_vecs, user_vecs, item_vecs, W,
         review_user_adj, review_item_adj, perm_u, perm_i,
         n_cores, rpc):
    n_users = user_vecs.shape[0]
    n_items = item_vecs.shape[0]
    n_rev = n_cores * rpc

    W = np.asarray(W, np.float32)
    perm_u = np.asarray(perm_u, np.int64)
    perm_i = np.asarray(perm_i, np.int64)
    W2p = np.empty_like(W[64:128])
    W2p[perm_u] = W[64:128]
    W3p = np.empty_like(W[128:192])
    W3p[perm_i] = W[128:192]
    w1b = np.ascontiguousarray(W[0:64].astype(ml_dtypes.bfloat16))
    w2d = np.ascontiguousarray(
        np.concatenate([W2p, W2p], axis=0).astype(ml_dtypes.bfloat16))
    w3b = np.ascontiguousarray(W3p.astype(ml_dtypes.bfloat16))

    user_vecs = np.asarray(user_vecs, np.float32)
    item_vecs = np.asarray(item_vecs, np.float32)
    tblu = _hilo(user_vecs)
    itemt = np.zeros((64, ITEM_PAD), ml_dtypes.bfloat16)
    itemt[:, :n_items] = item_vecs.T.astype(ml_dtypes.bfloat16)
    review_vecs = np.asarray(review_vecs, np.float32)[:n_rev]
    au_all = np.asarray(review_user_adj, np.int64)[:n_rev]
    ai_all = np.asarray(review_item_adj, np.int64)[:n_rev]

    # Global cell assignment: cell = (user-group, item-window); reviews in a
    # cell are dealt round-robin to cores so per-core counts are identical.
    g_all = au_all // TCH
    w_all = ai_all // WIN
    cell = g_all * NWIN + w_all
    order = np.argsort(cell, kind="stable")          # global review order
    counts = np.bincount(cell, minlength=NGRP * NWIN)
    m = (-(-counts // n_cores)).reshape(NGRP, NWIN)
    lay = Layout(m)
    padtot = lay.padtot

    nc = _get_program(lay, n_users)

    # position within cell for each sorted review
    within = np.arange(n_rev) - np.repeat(
        np.concatenate([[0], np.cumsum(counts)[:-1]]), counts)
    core_of = within % n_cores
    slot_of = within // n_cores
    cell_sorted = np.repeat(np.arange(NGRP * NWIN), counts)
    cb_flat = lay.cell_base.reshape(-1)
    col_of = cb_flat[cell_sorted] + slot_of  # column on its core

    # per-core inverse maps
    in_maps = []
    slotmaps = []
    # column -> group map (shared)
    colg = np.zeros(padtot, np.int64)
    for g in range(NGRP):
        colg[lay.grp_base[g]:lay.grp_base[g] + lay.grp_size[g]] = g
    # column -> window map (shared; padded cols -> -1)
    colw = np.full(padtot, -1, np.int64)
    for g in range(NGRP):
        for w in range(NWIN):
            b = lay.cell_base[g, w]
            colw[b:b + m[g, w]] = w

    for c in range(n_cores):
        sel = order[core_of == c]          # global review ids, cell-sorted
        cols = col_of[core_of == c]        # their columns
        slotmap = np.full(padtot, -1, np.int64)
        slotmap[cols] = sel
        valid = slotmap >= 0
        sl = np.where(valid, slotmap, 0)

        rt2 = np.ascontiguousarray(
            np.where(valid[:, None], review_vecs[sl], 0.0)
            .T.astype(ml_dtypes.bfloat16))
        u_reb = np.where(valid, au_all[sl] - colg * TCH, 0).astype(np.int16)
        oh = np.zeros((128, padtot), ml_dtypes.float8_e4m3)
        vc = np.nonzero(valid)[0]
        oh[(ai_all[slotmap[vc]] - colw[vc] * WIN), vc] = 1.0

        in_maps.append({
            "rt": rt2,
            "oh": oh,
            "uidx": _wrap_idx(u_reb, lay),
            "tblu": tblu,
            "itemt": itemt,
            "w1": w1b,
            "w2d": w2d,
            "w3p": w3b,
        })
        slotmaps.append((slotmap, valid))

    res = run_bass_kernel_spmd(nc, in_maps, core_ids=list(range(n_cores)))

    out = np.empty((n_rev, 64), np.float32)
    for c in range(n_cores):
        o2 = np.asarray(res.results[c]["out2"]).astype(np.float32)
        slotmap, valid = slotmaps[c]
        out[slotmap[valid]] = o2.T[valid]
    return out


def kernel(**inputs) -> np.ndarray:
    return _run(
        inputs["review_vecs"], inputs["user_vecs"], inputs["item_vecs"],
        inputs["W"], inputs["review_user_adj"], inputs["review_item_adj"],
        inputs["perm_u"], inputs["perm_i"],
        n_cores=N_CORES, rpc=RPC)


# revision 17
# speedup vs baseline: 2.6275x; 1.1157x over previous
"""Trainium2 Bass kernel for nn_ConcatenationAggregator.

For each review r:
    out[r] = relu(concat(review_vecs[r],
                         user_vecs[adj_u[r]][perm_u],
                         item_vecs[adj_i[r]][perm_i]) @ W)

Strategy (data-parallel over reviews, 8 NeuronCores):
  - Feature permutations are folded into W on the host (192x64 constant).
  - USER side: hi/lo-split bf16 table rows [bf16(x) | bf16(x - hi)] (256B)
    gathered by a transpose-mode GPSIMD dma_gather that delivers each row
    as a COLUMN across 128 partitions - already in matmul-rhs layout.
    lhsT = [W2p; W2p] makes the matmul compute W^T (hi + lo).
  - ITEM side: gather-free.  The device first computes T = item_vecs @ W3p
    (transform phase: stream item^T, 391 window matmuls, copy to an
    SBUF-resident bf16 table T_sb[p, w*64+j] = T[w*128+p, j]).  Reviews are
    globally sorted by (user-group, item-window); each 128-id item window
    becomes one accumulating matmul with lhsT = T_sb window and rhs = a
    streamed fp8 one-hot matrix (0/1 exact in fp8, 128B/review, dense DMA
    at full bandwidth - 4x cheaper than a 256B gather descriptor).
  - ONE program for all 8 cores: reviews in every (group, window) cell are
    dealt round-robin to cores, so per-core cell counts are the shared
    ceil(n_cell/8) - column layout and window-matmul ranges are identical
    across cores.  Padded slots have all-zero one-hot columns.
  - dma_gather needs int16 indices: 4 user groups of 32768 rows; the host
    rebases.  Transpose-mode gathers stage idx*256B through the 16KB DMA
    scratch -> max 896 indices per call (MAX_S = 7).
  - Review stream and output travel as bf16, host-transposed feature-major.
  - This toolchain build enforces ONE sync-wait slot per instruction:
    pool-reuse waits are absorbed by 1-column PE matmul warms / 1-element
    DVE memsets, and the kernel-tail drain is split into single-wait drains.
"""

import os
import types

import numpy as np
import ml_dtypes

import concourse.bacc as bacc
import concourse.bass as bass
import concourse.mybir as mybir
import concourse.tile as tile
from concourse.bass_utils import run_bass_kernel_spmd
from concourse.vector_clock import ScopedClock, VectorClock

F32 = mybir.dt.float32
BF16 = mybir.dt.bfloat16
FP8 = mybir.dt.float8e4
I16 = mybir.dt.int16

N_CORES = 8
D = 64
SUB = 128                  # reviews per sub-tile
MAX_S = 7                  # sub-tiles per chunk (transpose dma_gather stages
                           # idx*256B through the 16KB DMA scratch: 896 max)
TCH = 32768                # user table chunk (int16 index range)
PCOL = 512                 # PSUM tile columns
WIN = 64                   # item ids per window
NGRP = 4                   # user groups

N_REVIEWS = 1_000_000
N_USERS = 100_000
N_ITEMS = 50_000
RPC = N_REVIEWS // N_CORES
NWIN = -(-N_ITEMS // WIN)  # 782
ITEM_PAD = -(-NWIN * WIN // 1024) * 1024  # 50176 (transform pieces of 1024)

BUFS = int(os.environ.get("KBUFS", "3"))
PREF = int(os.environ.get("KPREF", "2"))
PSUM_BUFS = int(os.environ.get("KPSUM", "5"))


def _split_drain_and_barrier(self, tick_clock, wait_clock):
    """Replacement for TileContext._drain_and_barrier: the stock tail drain
    waits on every live proc semaphore at once, which overflows this
    toolchain's one-sync-wait-per-instruction limit.  Emit one drain per
    semaphore instead."""
    gc = tick_clock.global_clock
    ticks = list(gc)
    idxs = [i for i, t in enumerate(ticks) if t > 0]
    for i in idxs:
        sub = [0] * len(ticks)
        sub[i] = ticks[i]
        drain_inst = self.nc.sync.drain()
        wait_clock.add_sem_waits(
            drain_inst.ins, ScopedClock({None: VectorClock(sub)}))
    if not idxs:
        drain_inst = self.nc.sync.drain()
        wait_clock.add_sem_waits(
            drain_inst.ins, ScopedClock({None: VectorClock(ticks)}))
    self.nc.all_engine_barrier()
    assert self.sems is not None
    popped = self.nc._tile_sem_poison_stack.pop()
    assert popped is self._sem_poison
    self.nc.clear_and_free_semaphores(list(self.sems.allocated().values()))
    self.nc.all_engine_barrier()


class Layout:
    """Shared (all-cores) column layout derived from global cell counts.

    m[g, w]: padded per-core review count of cell (user-group g, window w).
    Group sections are padded to multiples of SUB; chunks of <= MAX_S
    sub-tiles tile each group section.
    """

    def __init__(self, m):
        self.m = m  # [NGRP, NWIN] ints
        self.cell_base = np.zeros((NGRP, NWIN), np.int64)
        self.grp_base = np.zeros(NGRP, np.int64)
        self.grp_size = np.zeros(NGRP, np.int64)  # padded
        pos = 0
        for g in range(NGRP):
            self.grp_base[g] = pos
            csum = 0
            for w in range(NWIN):
                self.cell_base[g, w] = pos + csum
                csum += m[g, w]
            padded = -(-csum // SUB) * SUB
            self.grp_size[g] = padded
            pos += padded
        self.padtot = int(pos)
        # chunks: (g, s, row)
        self.chunks = []
        for g in range(NGRP):
            left = int(self.grp_size[g]) // SUB
            row = int(self.grp_base[g])
            while left > 0:
                s = min(MAX_S, left)
                self.chunks.append((g, s, row))
                row += s * SUB
                left -= s
        # per chunk: list of psum tiles [(lo, hi)], and per tile the window
        # matmul splits [(lo, hi, w)] in ABSOLUTE columns
        self.tiles = []
        for (g, s, row) in self.chunks:
            tl = []
            for q in range(-(-s * 128 // PCOL)):
                lo = row + q * PCOL
                hi = min(row + s * 128, lo + PCOL)
                wmms = []
                for w in range(NWIN):
                    if self.m[g, w] == 0:
                        continue
                    clo = int(self.cell_base[g, w])
                    chi = clo + int(self.m[g, w])
                    a, b = max(clo, lo), min(chi, hi)
                    if a < b:
                        wmms.append((a, b, w))
                tl.append((lo, hi, wmms))
            self.tiles.append(tl)

    def key(self):
        return tuple(self.m.reshape(-1).tolist())


def _build_program(lay: Layout, n_users):
    nc = bacc.Bacc("TRN2", target_bir_lowering=False, debug=False,
                   enable_asserts=False)
    padtot = lay.padtot
    icols = padtot // 16

    rt_d = nc.dram_tensor("rt", [64, padtot], BF16, kind="ExternalInput")
    oh_d = nc.dram_tensor("oh", [64, padtot], FP8, kind="ExternalInput")
    uidx_d = nc.dram_tensor("uidx", [128, icols], I16, kind="ExternalInput")
    tblu_d = nc.dram_tensor("tblu", [n_users, 128], BF16,
                            kind="ExternalInput")
    itemt_d = nc.dram_tensor("itemt", [64, ITEM_PAD], BF16,
                             kind="ExternalInput")
    w1_d = nc.dram_tensor("w1", [64, 64], BF16, kind="ExternalInput")
    w2_d = nc.dram_tensor("w2d", [128, 64], BF16, kind="ExternalInput")
    w3_d = nc.dram_tensor("w3p", [64, 64], BF16, kind="ExternalInput")
    out_d = nc.dram_tensor("out2", [64, padtot], BF16, kind="ExternalOutput")

    T = len(lay.chunks)
    NPIECE = ITEM_PAD // 1024  # transform pieces of 8 windows
    ITILE = 4096               # itemT DMA span (4 pieces per load)
    NITILE = -(-ITEM_PAD // ITILE)

    # DMA spans: consecutive chunks, <= SPAN_CH chunks per rt/oh/out DMA.
    SPAN_CH = 4
    SPANMAX = SPAN_CH * MAX_S * 128
    spans = []  # (t0, t1, row0, ncols)
    t0 = 0
    while t0 < T:
        t1 = min(t0 + SPAN_CH, T)
        row0 = lay.chunks[t0][2]
        last_g, last_s, last_row = lay.chunks[t1 - 1]
        spans.append((t0, t1, row0, last_row + last_s * 128 - row0))
        t0 = t1
    NSP = len(spans)
    span_of = np.zeros(T, np.int64)
    for si, (a, b, _, _) in enumerate(spans):
        span_of[a:b] = si

    with tile.TileContext(nc) as tc:
        tc._drain_and_barrier = types.MethodType(_split_drain_and_barrier, tc)
        with tc.tile_pool(name="const", bufs=1) as constp, \
             tc.tile_pool(name="itp", bufs=2) as itp, \
             tc.tile_pool(name="rtp", bufs=BUFS) as rtp, \
             tc.tile_pool(name="ohp", bufs=BUFS) as ohp, \
             tc.tile_pool(name="gup", bufs=4 * BUFS) as gup, \
             tc.tile_pool(name="outp", bufs=BUFS) as outp, \
             tc.tile_pool(name="scr", bufs=1, space="PSUM") as scrp, \
             tc.tile_pool(name="tfp", bufs=2, space="PSUM") as tfp, \
             tc.tile_pool(name="mmp", bufs=PSUM_BUFS, space="PSUM") as mmp:

            uidx_t = constp.tile([128, icols], I16)
            nc.sync.dma_start(out=uidx_t[:], in_=uidx_d.ap()[:, :])
            w1_t = constp.tile([64, 64], BF16)
            nc.sync.dma_start(out=w1_t[:], in_=w1_d.ap()[:, :])
            w2_t = constp.tile([128, 64], BF16)
            nc.sync.dma_start(out=w2_t[:], in_=w2_d.ap()[:, :])
            w3_t = constp.tile([64, 64], BF16)
            nc.sync.dma_start(out=w3_t[:], in_=w3_d.ap()[:, :])
            tsb = constp.tile([64, ITEM_PAD], BF16)
            pscr = constp.tile([16, 16], I16)

            scratch = scrp.tile([64, PCOL], F32)
            # PE warmups: observe the weight loads (HWDGE sem), ramp p-state.
            nc.tensor.matmul(out=scratch[:, 0:64], lhsT=w1_t[:],
                             rhs=w1_t[:], start=True, stop=True)
            nc.tensor.matmul(out=scratch[:, 0:64], lhsT=w2_t[:],
                             rhs=w2_t[:, 0:64], start=True, stop=True)
            nc.tensor.matmul(out=scratch[:, 0:64], lhsT=w3_t[:],
                             rhs=w3_t[:], start=True, stop=True)
            # Pool warmup: observe the user index-table load.
            nc.gpsimd.tensor_copy(out=pscr[:, :], in_=uidx_t[0:16, 0:16])

            nreg = {}
            for (_, s, _) in lay.chunks:
                if s not in nreg:
                    nreg[s] = nc.gpsimd.to_reg(s * SUB)

            rt_tiles = [None] * NSP
            oh_tiles = [None] * NSP
            gu_tiles = [None] * T
            ps_tiles = [None] * T
            o_tiles = [None] * NSP

            def issue_span_loads(si):
                t0, t1, row0, ncols = spans[si]
                rt_t = rtp.tile([64, SPANMAX], BF16, tag="rt")
                nc.sync.dma_start(
                    out=rt_t[:, :ncols],
                    in_=rt_d.ap()[:, row0: row0 + ncols])
                oh_t = ohp.tile([64, SPANMAX], FP8, tag="oh")
                nc.sync.dma_start(
                    out=oh_t[:, :ncols],
                    in_=oh_d.ap()[:, row0: row0 + ncols])
                for t in range(t0, t1):
                    g, s, row = lay.chunks[t]
                    col = row // 16
                    gu_t = gup.tile([128, MAX_S * 128], BF16, tag="gu")
                    nc.gpsimd.dma_gather(
                        out_ap=gu_t[:, :s * 128].rearrange(
                            "p (n i) -> p n i", n=1),
                        in_ap=tblu_d.ap()[
                            g * TCH:min((g + 1) * TCH, n_users), :],
                        idxs_ap=uidx_t[:, col:col + s * 8],
                        num_idxs=s * SUB, num_idxs_reg=nreg[s],
                        elem_size=128, transpose=True)
                    gu_tiles[t] = gu_t
                rt_tiles[si], oh_tiles[si] = rt_t, oh_t

            for ss in range(min(PREF, NSP)):
                issue_span_loads(ss)

            # ---- transform phase: T_sb[p, w*64+j] = (item @ W3p)[w*128+p, j]
            # copies alternate DVE/ACT so the phase drains twice as fast
            itt = [None] * NITILE
            for k in range(NITILE):
                nco = min(ITILE, ITEM_PAD - k * ITILE)
                it_t = itp.tile([64, ITILE], BF16, tag="it")
                nc.sync.dma_start(
                    out=it_t[:, :nco],
                    in_=itemt_d.ap()[:, k * ITILE:k * ITILE + nco])
                itt[k] = it_t
            for pc in range(NPIECE):
                it_t = itt[pc // 4]
                co = (pc % 4) * 1024
                ps = tfp.tile([128, PCOL], F32, tag="tf")
                # 1-col warm write absorbs the PSUM-reuse wait.
                nc.tensor.matmul(out=ps[0:1, 0:1], lhsT=w3_t[:, 0:1],
                                 rhs=w3_t[:, 0:1], start=True, stop=True)
                for k in range(8):
                    nc.tensor.matmul(
                        out=ps[:, k * 64:(k + 1) * 64],
                        lhsT=it_t[:, co + k * 128:co + (k + 1) * 128],
                        rhs=w3_t[:], start=True, stop=True)
                dst = tsb[:, pc * 1024:(pc + 1) * 1024].rearrange(
                    "p (b c) -> p b c", c=64)
                lo_half = ps[0:64, :].rearrange("p (b c) -> p b c", c=64)
                hi_half = ps[64:128, :].rearrange("p (b c) -> p b c", c=64)
                if pc % 2 == 0:
                    nc.vector.tensor_copy(out=dst[:, 0::2, :], in_=lo_half)
                    nc.scalar.copy(out=dst[:, 1::2, :], in_=hi_half)
                else:
                    nc.scalar.copy(out=dst[:, 0::2, :], in_=lo_half)
                    nc.vector.tensor_copy(out=dst[:, 1::2, :], in_=hi_half)
            # PE observes T_sb (one matmul per copy engine, single waits):
            # later window matmuls elide the T_sb dependency.
            nc.tensor.matmul(out=scratch[:, 0:64], lhsT=tsb[:, 0:64],
                             rhs=w1_t[:], start=True, stop=True)
            nc.tensor.matmul(out=scratch[:, 0:64],
                             lhsT=tsb[:, ITEM_PAD - 64:ITEM_PAD],
                             rhs=w1_t[:], start=True, stop=True)

            def issue_matmuls(t):
                g, s, row = lay.chunks[t]
                si = span_of[t]
                row0 = spans[si][2]
                rt_t, oh_t, gu_t = rt_tiles[si], oh_tiles[si], gu_tiles[t]
                d0 = row - row0
                ps = mmp.tile([128, PCOL], F32, tag="mm")
                nc.tensor.matmul(out=ps[0:1, 0:1], lhsT=w2_t[:, 0:1],
                                 rhs=w2_t[:, 0:1], start=True, stop=True)
                for q, (lo, hi, wmms) in enumerate(lay.tiles[t]):
                    n = hi - lo
                    ph = ps[q * 64:(q + 1) * 64, :n]
                    c0 = lo - row
                    nc.tensor.matmul(out=ph, lhsT=w2_t[:],
                                     rhs=gu_t[:, c0:c0 + n],
                                     start=True, stop=False)
                    for (a, b, w) in wmms:
                        nc.tensor.matmul(
                            out=ps[q * 64:(q + 1) * 64, a - lo:b - lo],
                            lhsT=tsb[:, w * 64:(w + 1) * 64],
                            rhs=oh_t[:, d0 + a - row:d0 + b - row],
                            start=False, stop=False)
                    # last: full-range stop closes every column's group
                    # (rt's SP wait is covered by the oh wait).
                    nc.tensor.matmul(out=ph, lhsT=w1_t[:],
                                     rhs=rt_t[:, d0 + c0:d0 + c0 + n],
                                     start=False, stop=True)
                ps_tiles[t] = ps

            def issue_relus(t):
                g, s, row = lay.chunks[t]
                si = span_of[t]
                t0, _, row0, ncols = spans[si]
                if t == t0:
                    o_t = outp.tile([64, SPANMAX], BF16, tag="o")
                    nc.vector.memset(o_t[0:1, 0:1], 0)
                    o_tiles[si] = o_t
                o_t = o_tiles[si]
                ps = ps_tiles[t]
                for q, (lo, hi, _) in enumerate(lay.tiles[t]):
                    n = hi - lo
                    nc.vector.tensor_scalar_max(
                        out=o_t[:, lo - row0:lo - row0 + n],
                        in0=ps[q * 64:(q + 1) * 64, :n], scalar1=0.0)

            def issue_store(si):
                _, _, row0, ncols = spans[si]
                nc.sync.dma_start(
                    out=out_d.ap()[:, row0: row0 + ncols],
                    in_=o_tiles[si][:, :ncols])

            for si in range(NSP):
                if si + PREF < NSP:
                    issue_span_loads(si + PREF)
                for t in range(spans[si][0], spans[si][1]):
                    issue_matmuls(t)
                    issue_relus(t)
                issue_store(si)
    nc.finalize()
    return nc


_PROGRAM_CACHE: dict = {}


def _get_program(lay: Layout, n_users):
    key = (lay.key(), n_users)
    if key not in _PROGRAM_CACHE:
        _PROGRAM_CACHE[key] = _build_program(lay, n_users)
    return _PROGRAM_CACHE[key]


def _wrap_idx(flat_sorted, lay):
    """Rebased int16 indices [PADTOT] -> [128, PADTOT//16] in dma_gather's
    wrapped layout (per chunk: [p, m] = flat[m*16 + p%16], replicated)."""
    cols = []
    for (_, s, row) in lay.chunks:
        blk = flat_sorted[row:row + s * SUB].reshape(s * 8, 16).T
        cols.append(np.tile(blk, (8, 1)))
    return np.ascontiguousarray(np.concatenate(cols, axis=1))


def _hilo(tbl):
    """[N, 64] f32 -> [N, 128] bf16 rows [hi | lo]."""
    hi = tbl.astype(ml_dtypes.bfloat16)
    lo = (tbl - hi.astype(np.float32)).astype(ml_dtypes.bfloat16)
    return np.ascontiguousarray(np.concatenate([hi, lo], axis=1))


def _run(review_vecs, user_vecs, item_vecs, W,
         review_user_adj, review_item_adj, perm_u, perm_i,
         n_cores, rpc):
    n_users = user_vecs.shape[0]
    n_items = item_vecs.shape[0]
    n_rev = n_cores * rpc

    W = np.asarray(W, np.float32)
    perm_u = np.asarray(perm_u, np.int64)
    perm_i = np.asarray(perm_i, np.int64)
    W2p = np.empty_like(W[64:128])
    W2p[perm_u] = W[64:128]
    W3p = np.empty_like(W[128:192])
    W3p[perm_i] = W[128:192]
    w1b = np.ascontiguousarray(W[0:64].astype(ml_dtypes.bfloat16))
    w2d = np.ascontiguousarray(
        np.concatenate([W2p, W2p], axis=0).astype(ml_dtypes.bfloat16))
    w3b = np.ascontiguousarray(W3p.astype(ml_dtypes.bfloat16))

    user_vecs = np.asarray(user_vecs, np.float32)
    item_vecs = np.asarray(item_vecs, np.float32)
    tblu = _hilo(user_vecs)
    itemt = np.zeros((64, ITEM_PAD), ml_dtypes.bfloat16)
    itemt[:, :n_items] = item_vecs.T.astype(ml_dtypes.bfloat16)
    review_vecs = np.asarray(review_vecs, np.float32)[:n_rev]
    au_all = np.asarray(review_user_adj, np.int64)[:n_rev]
    ai_all = np.asarray(review_item_adj, np.int64)[:n_rev]

    # Global cell assignment: cell = (user-group, item-window); reviews in a
    # cell are dealt round-robin to cores so per-core counts are identical.
    g_all = au_all // TCH
    w_all = ai_all // WIN
    cell = g_all * NWIN + w_all
    order = np.argsort(cell, kind="stable")          # global review order
    counts = np.bincount(cell, minlength=NGRP * NWIN)
    m = (-(-counts // n_cores)).reshape(NGRP, NWIN)
    lay = Layout(m)
    padtot = lay.padtot

    nc = _get_program(lay, n_users)

    # position within cell for each sorted review
    within = np.arange(n_rev) - np.repeat(
        np.concatenate([[0], np.cumsum(counts)[:-1]]), counts)
    core_of = within % n_cores
    slot_of = within // n_cores
    cell_sorted = np.repeat(np.arange(NGRP * NWIN), counts)
    cb_flat = lay.cell_base.reshape(-1)
    col_of = cb_flat[cell_sorted] + slot_of  # column on its core

    # per-core inverse maps
    in_maps = []
    slotmaps = []
    # column -> group map (shared)
    colg = np.zeros(padtot, np.int64)
    for g in range(NGRP):
        colg[lay.grp_base[g]:lay.grp_base[g] + lay.grp_size[g]] = g
    # column -> window map (shared; padded cols -> -1)
    colw = np.full(padtot, -1, np.int64)
    for g in range(NGRP):
        for w in range(NWIN):
            b = lay.cell_base[g, w]
            colw[b:b + m[g, w]] = w

    for c in range(n_cores):
        sel = order[core_of == c]          # global review ids, cell-sorted
        cols = col_of[core_of == c]        # their columns
        slotmap = np.full(padtot, -1, np.int64)
        slotmap[cols] = sel
        valid = slotmap >= 0
        sl = np.where(valid, slotmap, 0)

        rt2 = np.ascontiguousarray(
            np.where(valid[:, None], review_vecs[sl], 0.0)
            .T.astype(ml_dtypes.bfloat16))
        u_reb = np.where(valid, au_all[sl] - colg * TCH, 0).astype(np.int16)
        oh = np.zeros((64, padtot), ml_dtypes.float8_e4m3)
        vc = np.nonzero(valid)[0]
        oh[(ai_all[slotmap[vc]] - colw[vc] * WIN), vc] = 1.0

        in_maps.append({
            "rt": rt2,
            "oh": oh,
            "uidx": _wrap_idx(u_reb, lay),
            "tblu": tblu,
            "itemt": itemt,
            "w1": w1b,
            "w2d": w2d,
            "w3p": w3b,
        })
        slotmaps.append((slotmap, valid))

    res = run_bass_kernel_spmd(nc, in_maps, core_ids=list(range(n_cores)))

    out = np.empty((n_rev, 64), np.float32)
    for c in range(n_cores):
        o2 = np.asarray(res.results[c]["out2"]).astype(np.float32)
        slotmap, valid = slotmaps[c]
        out[slotmap[valid]] = o2.T[valid]
    return out


def kernel(**inputs) -> np.ndarray:
    return _run(
        inputs["review_vecs"], inputs["user_vecs"], inputs["item_vecs"],
        inputs["W"], inputs["review_user_adj"], inputs["review_item_adj"],
        inputs["perm_u"], inputs["perm_i"],
        n_cores=N_CORES, rpc=RPC)
